# revision 1
# baseline (speedup 1.0000x reference)
"""Trainium2 Bass kernel for the DGL-JTNN tree decoder (nn_DGLJTNNDecoder).

Strategy (see problem sharding hint): pure data-parallel over the 512 trees,
64 trees per NeuronCore. Per core the T=38-step GRU message-passing scan is
computed with:
  - edges relabeled by DFS step -> state writes are contiguous, state lives in
    an append-only DRAM log whose per-step gather reads a strict PREFIX (so
    Tile's byte-range dependency tracker lets gathers of step t overlap writes
    of steps t-1/t-2),
  - all embedding-dependent matmul terms folded into host-precomputed tables
    (Ez = emb@Wz[:H] + bz etc.), fetched per step with dma_gather,
  - dma_gather(transpose=True) producing feature-major [512, n] operands
    directly, fp16 state + fp16 matmuls with fp32 PSUM accumulation,
  - contributions from steps t-1/t-2 applied via masked adds from SBUF-resident
    m/rm tiles (one edge per tree per step -> same column = same tree).
The two MLP heads run as large fused feature-major matmuls over the SBUF-
resident h-slab; losses/accuracies are reduced on-device to 4 partial sums per
core and combined on the host (the "loss all-reduce").
"""

import numpy as np
import ml_dtypes

import concourse.bass as bass
import concourse.bacc as bacc
import concourse.mybir as mybir
import concourse.tile as tile
from concourse.library_config import mlp as _mlp_lib
from concourse.bass_utils import run_bass_kernel_spmd

f16 = mybir.dt.float16
f32 = mybir.dt.float32
i16 = mybir.dt.int16
i32 = mybir.dt.int32
AF = mybir.ActivationFunctionType
ALU = mybir.AluOpType

# problem constants (hardcoded per contract)
B, N, H, L, V = 512, 20, 450, 56, 780
T = 2 * (N - 1)            # 38 steps
EpT = T                    # edges per tree
NC = 8                     # cores
C = B // NC                # 64 trees/core
Hp = 512                   # padded hidden
NBLK = 40                  # head col blocks (39 real + 1 pad) -> 2560 cols
NCOL = NBLK * C            # 2560
RC = NCOL // 128           # 20 row chunks
P, Dn = 3, 4
LOG_ROWS = 64 + T * 128    # state log rows (64 zero sentinels first)
ME_ROWS = 64 + 4 * 1024    # megaE rows: 64 zeros + Ez|Eh|Er|Eu
NI_S = 640                 # state gather idxs/step
NI_E = 256                 # E gather idxs/step

import os
DBG_T = int(os.environ.get("KDBG_T", T))        # scan steps to emit
DBG_HEADS = os.environ.get("KDBG_HEADS", "1") == "1"
DBG_Q = os.environ.get("KDBG_Q", "1") == "1"
DBG_FIN = os.environ.get("KDBG_FIN", "1") == "1"
DBG_QOPS = int(os.environ.get("KDBG_QOPS", "9")) 


def _wrap_idx(idx):
    """[n*16] flat gather order -> [16, n] wrapped, replicated to 128 rows."""
    idx = np.asarray(idx, np.int16)
    n = idx.shape[0] // 16
    return np.tile(idx.reshape(n, 16).T, (8, 1))    # [128, n]


def _host_prep(inputs):
    inp = {k: np.asarray(v) for k, v in inputs.items()}
    (tree_vec, emb, Wz, bz, Wh, bh, Wr, Ur, br, Ww, bw, Uw, bu, Wo, bo,
     Us, bs) = (inp[k] for k in
                ['tree_vec', 'emb', 'Wz', 'bz', 'Wh', 'bh', 'Wr', 'Ur', 'br',
                 'Ww', 'bw', 'Uw', 'bu', 'Wo', 'bo', 'Us', 'bs'])
    wid, root_ids = inp['wid'], inp['root_ids']
    edge_src, edge_dst = inp['edge_src'], inp['edge_dst']
    edge_pred, node_in = inp['edge_pred'], inp['node_in']
    step_eid, step_v = inp['step_eid'], inp['step_v']
    q_rows, q_tgt, p_tgt = inp['q_rows'], inp['q_tgt'], inp['p_tgt']
    n_edges = edge_src.shape[0]

    def padHp(M, axis):
        pads = [(0, 0)] * M.ndim
        pads[axis] = (0, Hp - M.shape[axis])
        return np.pad(M, pads)

    def h16(x):
        return np.ascontiguousarray(x.astype(np.float16))

    # ---- folded tables (fp32 math, fp16 upload) ----
    Ez = padHp(emb @ Wz[:H] + bz, 1)
    Eh = padHp(emb @ Wh[:H] + bh, 1)
    Er = padHp(emb @ Wr + br, 1)
    Eu = padHp(emb @ Uw[:H] + bu, 1)
    Eu[:, 511] = 1.0                      # bias-injection row for p-head (bs)
    megaE = np.zeros((ME_ROWS, Hp), np.float32)
    for i, E in enumerate([Ez, Eh, Er, Eu]):
        megaE[64 + i * 1024: 64 + i * 1024 + V] = E
    megaE = h16(megaE)

    def packW(Wm):                         # [512,512] -> [128, 4*512]
        return h16(Wm.reshape(4, 128, Hp).transpose(1, 0, 2).reshape(128, 4 * Hp))

    wz_h = packW(padHp(padHp(Wz[H:], 0), 1))
    wh_h = packW(padHp(padHp(Wh[H:], 0), 1))
    wu_h = packW(padHp(padHp(Ur, 0), 1))
    wuwh_h = packW(padHp(padHp(Uw[H:2 * H], 0), 1))
    wwwh_h = packW(padHp(padHp(Ww[:H], 0), 1))
    Wop = padHp(Wo, 0)
    Wop[511, :] = bo                      # bias row (qrelu[511]==1)
    wo_h = h16(Wop.reshape(4, 128, V).transpose(1, 0, 2).reshape(128, 4 * V))
    Usp = padHp(Us, 0)
    Usp[511, 0] = bs[0]                   # bias row (prelu[511]==1)
    us_h = h16(Usp.reshape(4, 128).T)     # [128, 4]

    # tree_vec padded with ones column (bias channel)
    tvpad = np.zeros((B, 128), np.float32)
    tvpad[:, :L] = tree_vec
    tvpad[:, L] = 1.0
    tvpad = h16(tvpad)
    WuL = np.zeros((64, Hp), np.float32)
    WuL[:L] = padHp(Uw[2 * H:], 1)
    wul_h = h16(WuL)
    WwL = np.zeros((64, Hp), np.float32)
    WwL[:L] = padHp(Ww[H:], 1)
    WwL[L, :H] = bw                       # ones channel -> +bw
    WwL[L, 511] = 1.0                     # makes T_w[:,511]=1 -> qrelu[511]=1
    wwl_h = h16(WwL)

    # ---- per-core graph metadata ----
    estep = np.full(n_edges, -1, np.int64)
    for t in range(T):
        for b in range(B):
            estep[step_eid[t, b]] = t

    cores = []
    for core in range(NC):
        trees = np.arange(core * C, (core + 1) * C)
        sidx = np.zeros((T, NI_S), np.int32)          # -> row 0 sentinel default
        eidx = np.zeros((T, NI_E), np.int32)
        masks = np.zeros((T, 384), np.float16)        # P1|P2|N1x64|N2x64 (P* are 128 wide)
        for t in range(T):
            for j, b in enumerate(trees):
                e = step_eid[t, b]
                v = step_v[t, b]
                kk = 0
                for p in range(P):
                    pe = edge_pred[e, p]
                    if pe >= n_edges:
                        continue
                    tp = estep[pe]
                    if tp > t:
                        continue
                    if tp == t - 1:
                        masks[t, j] = 1.0; masks[t, 64 + j] = 1.0
                    elif tp == t - 2:
                        masks[t, 128 + j] = 1.0; masks[t, 192 + j] = 1.0
                    else:
                        sidx[t, kk * 128 + j] = 64 + tp * 128 + j
                        sidx[t, kk * 128 + 64 + j] = 64 + tp * 128 + 64 + j
                        kk += 1
                kk = 0
                for p in range(Dn):
                    ie = node_in[v, p]
                    if ie >= n_edges or ie == e:
                        continue
                    ti = estep[ie]
                    if ti > t:
                        continue
                    if ti == t - 1:
                        masks[t, 256 + j] = 1.0
                    elif ti == t - 2:
                        masks[t, 320 + j] = 1.0
                    else:
                        sidx[t, 384 + kk * 64 + j] = 64 + ti * 128 + j
                        kk += 1
                ws = wid[edge_src[e]]
                wd = wid[edge_dst[e]]
                eidx[t, j] = 64 + ws
                eidx[t, 64 + j] = 64 + 1024 + ws
                eidx[t, 128 + j] = 64 + 2048 + wd
                eidx[t, 192 + j] = 64 + 3072 + wid[step_v[t, b]]

        # assert prefix property
        for t in range(T):
            assert sidx[t].max() < 64 + max(0, t - 2) * 128

        # head E_u gather: col = k*64+j (k=0 root, k=t+1 step t); 5 cc x 512
        widrow = np.zeros(NCOL, np.int32)
        for j, b in enumerate(trees):
            widrow[j] = wid[root_ids[b]]
        for t in range(T):
            for j, b in enumerate(trees):
                widrow[(t + 1) * 64 + j] = wid[step_v[t, b]]
        uidx = (64 + 3072 + widrow).astype(np.int32)  # pad cols -> Eu[0] finite

        # q/p loss tables, row-major [128, 20]: local row l = rc*128 + p
        qmask = np.zeros((128, RC), np.float32)
        qtg = np.zeros((128, RC), np.float32)
        ptgt = np.zeros((128, RC), np.float32)
        pmask = np.zeros((128, RC), np.float32)
        for i in range(q_rows.shape[0]):
            g = int(q_rows[i])
            k, b = g // B, g % B
            if core * C <= b < (core + 1) * C:
                l = k * C + (b - core * C)
                qmask[l % 128, l // 128] = 1.0
                qtg[l % 128, l // 128] = float(q_tgt[i])
        for l in range(39 * C):
            k, j = l // C, l % C
            g = k * B + core * C + j
            ptgt[l % 128, l // 128] = float(p_tgt[g])
            pmask[l % 128, l // 128] = 1.0

        sidx_w = np.zeros((128, T, NI_S // 16), np.int16)
        eidx_w = np.zeros((128, T, NI_E // 16), np.int16)
        for t in range(T):
            sidx_w[:, t, :] = _wrap_idx(sidx[t])
            eidx_w[:, t, :] = _wrap_idx(eidx[t])
        uidx_w = np.zeros((128, 5, 32), np.int16)
        for cc in range(5):
            uidx_w[:, cc, :] = _wrap_idx(uidx[cc * 512:(cc + 1) * 512])
        tvidx = np.zeros(128, np.int32)
        tvidx[:C] = trees
        tvidx_w = _wrap_idx(tvidx)

        masks_r = np.broadcast_to(masks.reshape(1, T * 384), (128, T * 384))

        cores.append(dict(
            megaE=megaE, tvpad=tvpad,
            wz=wz_h, wh=wh_h, wu=wu_h, wuwh=wuwh_h, wwwh=wwwh_h,
            wo=wo_h, us=us_h, wul=wul_h, wwl=wwl_h,
            sidx=np.ascontiguousarray(sidx_w.reshape(128, T * (NI_S // 16))),
            eidx=np.ascontiguousarray(eidx_w.reshape(128, T * (NI_E // 16))),
            uidx=np.ascontiguousarray(uidx_w.reshape(128, 5 * 32)),
            tvidx=np.ascontiguousarray(tvidx_w),
            masks=np.ascontiguousarray(masks_r, ).astype(np.float16),
            qtg=qtg, qmask=qmask, ptgt=ptgt, pmask=pmask,
            iota=np.broadcast_to(np.arange(V, dtype=np.float32), (128, V)).copy(),
        ))
    return cores


def _build_program():
    nc = bacc.Bacc("TRN2", debug=False)

    D = {}
    def di(name, shape, dt):
        D[name] = nc.dram_tensor(name, shape, dt, kind="ExternalInput")
        return D[name]

    di("megaE", [ME_ROWS, Hp], f16)
    di("tvpad", [B, 128], f16)
    for w in ["wz", "wh", "wu", "wuwh", "wwwh"]:
        di(w, [128, 4 * Hp], f16)
    di("wo", [128, 4 * V], f16)
    di("us", [128, 4], f16)
    di("wul", [64, Hp], f16)
    di("wwl", [64, Hp], f16)
    di("sidx", [128, T * (NI_S // 16)], i16)
    di("eidx", [128, T * (NI_E // 16)], i16)
    di("uidx", [128, 5 * 32], i16)
    di("tvidx", [128, 8], i16)
    di("masks", [128, T * 384], f16)
    for x in ["qtg", "qmask", "ptgt", "pmask"]:
        di(x, [128, RC], f32)
    di("iota", [128, V], f32)
    out_d = nc.dram_tensor("out", [1, 8], f32, kind="ExternalOutput")

    mlog = nc.dram_tensor("mlog", [LOG_ROWS, Hp], f16, kind="Internal")
    pl_dram = nc.dram_tensor("pl_scratch", [1, NCOL], f32, kind="Internal")

    with tile.TileContext(nc) as tc:
        with tc.tile_pool(name="const", bufs=1) as cp:
            nc.gpsimd.load_library(_mlp_lib)
            rg = {n: nc.gpsimd.to_reg(n) for n in (128, NI_E, 512, NI_S)}
            # ---- load constants ----
            def ld(name, shape, dt):
                t_ = cp.tile(shape, dt, tag=name)
                nc.sync.dma_start(out=t_[:], in_=D[name][:].rearrange(
                    "p (a b) -> p a b", a=shape[1]) if len(shape) == 3 else D[name][:])
                return t_
            wz = ld("wz", [128, 4, Hp], f16)
            wh = ld("wh", [128, 4, Hp], f16)
            wu = ld("wu", [128, 4, Hp], f16)
            wuwh = ld("wuwh", [128, 4, Hp], f16)
            wwwh = ld("wwwh", [128, 4, Hp], f16)
            wo = ld("wo", [128, 4, V], f16)
            us = ld("us", [128, 4], f16)
            wul = cp.tile([64, Hp], f16)
            nc.sync.dma_start(out=wul[:], in_=D["wul"][:])
            wwl = cp.tile([64, Hp], f16)
            nc.sync.dma_start(out=wwl[:], in_=D["wwl"][:])
            sidx = ld("sidx", [128, T, NI_S // 16], i16)
            eidx = ld("eidx", [128, T, NI_E // 16], i16)
            uidx = ld("uidx", [128, 5, 32], i16)
            tvidx = cp.tile([128, 8], i16)
            nc.sync.dma_start(out=tvidx[:], in_=D["tvidx"][:])
            masks = ld("masks", [128, T, 384], f16)
            qtg = cp.tile([128, RC], f32)
            nc.sync.dma_start(out=qtg[:], in_=D["qtg"][:])
            qmask = cp.tile([128, RC], f32)
            nc.sync.dma_start(out=qmask[:], in_=D["qmask"][:])
            ptgt = cp.tile([128, RC], f32)
            nc.sync.dma_start(out=ptgt[:], in_=D["ptgt"][:])
            pmask = cp.tile([128, RC], f32)
            nc.sync.dma_start(out=pmask[:], in_=D["pmask"][:])

            ident = cp.tile([128, 128], f16)
            from concourse.masks import make_identity
            make_identity(nc, ident[:])
            iota_f = cp.tile([128, V], f32, tag="iota")
            nc.sync.dma_start(out=iota_f[:], in_=D["iota"][:])

            zt64 = cp.tile([64, Hp], f16)
            nc.vector.memset(zt64[:], 0.0)
            nc.sync.dma_start(out=mlog[0:64, :], in_=zt64[:])

            # h slab (feature-major), zeroed (roots + pad cols)
            hslab = cp.tile([128, 4, NCOL], f16)
            nc.vector.memset(hslab[:], 0.0)

            # T_u / T_w from tree_vec
            with tc.tile_pool(name="init_ps", bufs=1, space="PSUM") as ipp:
                tvg = cp.tile([128, 1, 128], f16)
                nc.gpsimd.dma_gather(tvg[:], D["tvpad"][:], tvidx[:], 128,
                                     rg[128], 128, transpose=True)
                tun = cp.tile([128, 4, C], f16)
                twn = cp.tile([128, 4, C], f16)
                for dst, wl in ((tun, wul), (twn, wwl)):
                    ps = ipp.tile([128, 4, C], f32, space="PSUM")
                    for m in range(4):
                        nc.tensor.matmul(ps[:, m, :], lhsT=wl[0:64, bass.ts(m, 128)],
                                         rhs=tvg[0:64, 0, 0:C], start=True, stop=True)
                    nc.vector.tensor_copy(dst[:], ps[:])

            # ---------------- scan ----------------
            with tc.tile_pool(name="sg", bufs=4) as sgp, \
                 tc.tile_pool(name="eg", bufs=4) as egp, \
                 tc.tile_pool(name="wk", bufs=2) as wkp, \
                 tc.tile_pool(name="mrm", bufs=3) as mrmp, \
                 tc.tile_pool(name="scps", bufs=2, space="PSUM") as scps:
                M_hist = []
                for t in range(DBG_T):
                    Xt = 64 + max(0, t - 2) * 128
                    sg = sgp.tile([128, 4, NI_S], f16, tag="sg")
                    nc.gpsimd.dma_gather(sg[:], mlog[0:Xt, :], sidx[:, t, :],
                                         NI_S, rg[NI_S], Hp, transpose=True)
                    eg = egp.tile([128, 4, NI_E], f16, tag="eg")
                    nc.gpsimd.dma_gather(eg[:], D["megaE"][:], eidx[:, t, :],
                                         NI_E, rg[NI_E], Hp, transpose=True)

                    S = wkp.tile([128, 4, 128], f16, tag="S")
                    nc.any.tensor_add(S[:], sg[:, :, 0:128], sg[:, :, 128:256])
                    nc.any.tensor_add(S[:], S[:], sg[:, :, 256:384])
                    hp = wkp.tile([128, 4, 64], f32, tag="hp")
                    nc.any.tensor_add(hp[:], sg[:, :, 384:448], sg[:, :, 448:512])
                    nc.any.tensor_add(hp[:], hp[:], sg[:, :, 512:576])
                    for lag in (1, 2):
                        if t - lag < 0:
                            continue
                        Mp = M_hist[t - lag]
                        lgP = wkp.tile([128, 4, 128], f16, tag="lgP")
                        nc.any.tensor_mul(lgP[:], Mp[:],
                                          masks[:, t, bass.ts(lag - 1, 128)]
                                          .rearrange("p (o n) -> p o n", o=1)
                                          .to_broadcast([128, 4, 128]))
                        nc.any.tensor_add(S[:], S[:], lgP[:])
                        lgN = wkp.tile([128, 4, 64], f32, tag="lgN")
                        nc.any.tensor_mul(lgN[:], Mp[:, :, 0:64],
                                          masks[:, t, 256 + (lag - 1) * 64:
                                                256 + lag * 64]
                                          .rearrange("p (o n) -> p o n", o=1)
                                          .to_broadcast([128, 4, 64]))
                        nc.any.tensor_add(hp[:], hp[:], lgN[:])

                    zh_ps = scps.tile([128, 8, 64], f32, space="PSUM", tag="zh")
                    for m in range(4):
                        for k in range(4):
                            nc.tensor.matmul(zh_ps[:, m, :],
                                             lhsT=wz[:, k, bass.ts(m, 128)],
                                             rhs=S[:, k, 0:64],
                                             start=(k == 0), stop=(k == 3))
                        for k in range(4):
                            nc.tensor.matmul(zh_ps[:, 4 + m, :],
                                             lhsT=wh[:, k, bass.ts(m, 128)],
                                             rhs=S[:, k, 64:128],
                                             start=(k == 0), stop=(k == 3))
                    zhs = wkp.tile([128, 8, 64], f32, tag="zhs")
                    nc.any.tensor_add(
                        zhs[:].rearrange("p (i c) f -> p i c f", i=2),
                        zh_ps[:].rearrange("p (i c) f -> p i c f", i=2),
                        eg[:, :, 0:128].rearrange("p c (i f) -> p i c f", i=2))
                    z_t = wkp.tile([128, 4, 64], f16, tag="z")
                    nc.scalar.activation(z_t[:], zhs[:, 0:4, :], AF.Sigmoid)
                    th_t = wkp.tile([128, 4, 64], f16, tag="th")
                    nc.scalar.activation(th_t[:], zhs[:, 4:8, :], AF.Tanh)

                    Mt = mrmp.tile([128, 4, 128], f16, tag="M")
                    d_t = wkp.tile([128, 4, 64], f32, tag="d")
                    nc.any.tensor_sub(d_t[:], th_t[:], S[:, :, 0:64])
                    nc.any.tensor_mul(d_t[:], z_t[:], d_t[:])
                    nc.any.tensor_add(Mt[:, :, 0:64], d_t[:], S[:, :, 0:64])

                    r_ps = scps.tile([128, 4, 64], f32, space="PSUM", tag="r")
                    for m in range(4):
                        for k in range(4):
                            nc.tensor.matmul(r_ps[:, m, :],
                                             lhsT=wu[:, k, bass.ts(m, 128)],
                                             rhs=Mt[:, k, 0:64],
                                             start=(k == 0), stop=(k == 3))
                    rs = wkp.tile([128, 4, 64], f32, tag="rs")
                    nc.any.tensor_add(rs[:], r_ps[:], eg[:, :, 128:192])
                    r_t = wkp.tile([128, 4, 64], f16, tag="rt")
                    nc.scalar.activation(r_t[:], rs[:], AF.Sigmoid)
                    nc.any.tensor_mul(Mt[:, :, 64:128], r_t[:], Mt[:, :, 0:64])

                    nc.any.tensor_add(hslab[:, :, bass.ts(t + 1, 64)],
                                      hp[:], Mt[:, :, 0:64])

                    tpm = scps.tile([64, 4, 128], f16, space="PSUM", tag="tpm")
                    tpr = scps.tile([64, 4, 128], f16, space="PSUM", tag="tpr")
                    for c in range(4):
                        nc.tensor.transpose(tpm[:, c, :], Mt[:, c, 0:64], ident[:])
                        nc.tensor.transpose(tpr[:, c, :], Mt[:, c, 64:128], ident[:])
                    stm = wkp.tile([64, 512], f16, tag="stm")
                    str_ = wkp.tile([64, 512], f16, tag="str")
                    nc.any.tensor_copy(stm[:], tpm[:].rearrange("p a b -> p (a b)"))
                    nc.any.tensor_copy(str_[:], tpr[:].rearrange("p a b -> p (a b)"))
                    base = 64 + t * 128
                    nc.sync.dma_start(out=mlog[base:base + 64, :], in_=stm[:])
                    nc.sync.dma_start(out=mlog[base + 64:base + 128, :], in_=str_[:])

                    M_hist.append(Mt)

            # ---------------- heads ----------------
            acc = cp.tile([128, 8], f32)
            nc.vector.memset(acc[:], 0.0)
            pl_sb = cp.tile([1, NCOL], f32)
            tl_all = cp.tile([128, RC], f32)
            mxn_all = cp.tile([128, RC], f32)
            se_all = cp.tile([128, RC], f32)

            # p-phase
            if not DBG_HEADS:
                nc.any.tensor_copy(acc[:, 0:1], hslab[:, 0, 0:1])
            if DBG_HEADS:
              with tc.tile_pool(name="php", bufs=1, space="PSUM") as php, \
                   tc.tile_pool(name="plps", bufs=2, space="PSUM") as plps, \
                   tc.tile_pool(name="pwk", bufs=2) as pwk, \
                   tc.tile_pool(name="eug", bufs=2) as eugp:
                  for cc in range(5):
                      cs = slice(cc * 512, (cc + 1) * 512)
                      eug = eugp.tile([128, 4, 512], f16, tag="eug")
                      nc.gpsimd.dma_gather(eug[:], D["megaE"][:], uidx[:, cc, :],
                                           512, rg[512], Hp, transpose=True)
                      pp = php.tile([128, 4, 512], f32, space="PSUM", tag="pp")
                      for m in range(4):
                          for k in range(4):
                              nc.tensor.matmul(pp[:, m, :],
                                               lhsT=wuwh[:, k, bass.ts(m, 128)],
                                               rhs=hslab[:, k, cs],
                                               start=(k == 0), stop=(k == 3))
                      ppre = pwk.tile([128, 4, 512], f16, tag="ppre")
                      nc.any.tensor_add(ppre[:], pp[:], eug[:])
                      nc.any.tensor_add(
                          ppre[:].rearrange("p c (i n) -> p c i n", i=8),
                          ppre[:].rearrange("p c (i n) -> p c i n", i=8),
                          tun[:].rearrange("p c (o n) -> p c o n", o=1)
                          .to_broadcast([128, 4, 8, C]))
                      nc.scalar.activation(ppre[:], ppre[:], AF.Relu)
                      pl_ps = plps.tile([1, 512], f32, space="PSUM", tag="pl")
                      for k in range(4):
                          nc.tensor.matmul(pl_ps[:], lhsT=us[:, k:k + 1],
                                           rhs=ppre[:, k, :],
                                           start=(k == 0), stop=(k == 3))
                      nc.any.tensor_copy(pl_sb[0:1, cs], pl_ps[:])

              # q-phase
              if DBG_Q:
               with tc.tile_pool(name="qhp", bufs=1, space="PSUM") as qhp, \
                    tc.tile_pool(name="qlps", bufs=2, space="PSUM") as qlps, \
                    tc.tile_pool(name="qwk", bufs=2) as qwk:
                   for cc in range(5):
                       cs = slice(cc * 512, (cc + 1) * 512)
                       qp = qhp.tile([128, 4, 512], f32, space="PSUM", tag="qp")
                       for m in range(4):
                           for k in range(4):
                               nc.tensor.matmul(qp[:, m, :],
                                                lhsT=wwwh[:, k, bass.ts(m, 128)],
                                                rhs=hslab[:, k, cs],
                                                start=(k == 0), stop=(k == 3))
                       qpre = qwk.tile([128, 4, 512], f16, tag="qpre")
                       nc.any.tensor_add(
                           qpre[:].rearrange("p c (i n) -> p c i n", i=8),
                           qp[:].rearrange("p c (i n) -> p c i n", i=8),
                           twn[:].rearrange("p c (o n) -> p c o n", o=1)
                           .to_broadcast([128, 4, 8, C]))
                       nc.scalar.activation(qpre[:], qpre[:], AF.Relu)
                       for rr in range(4):
                           rc = cc * 4 + rr
                           ql = qlps.tile([128, V], f32, space="PSUM", tag="ql")
                           for k in range(4):
                               nc.tensor.matmul(ql[:, 0:512],
                                                lhsT=qpre[:, k, bass.ts(rr, 128)],
                                                rhs=wo[:, k, 0:512],
                                                start=(k == 0), stop=(k == 3))
                           for k in range(4):
                               nc.tensor.matmul(ql[:, 512:V],
                                                lhsT=qpre[:, k, bass.ts(rr, 128)],
                                                rhs=wo[:, k, 512:V],
                                                start=(k == 0), stop=(k == 3))
                           nc.vector.tensor_reduce(out=mxn_all[:, rc:rc + 1],
                                                   in_=ql[:], op=ALU.max,
                                                   axis=mybir.AxisListType.X,
                                                   negate=True)
                           oh = qwk.tile([128, V], f32, tag="oh")
                           nc.vector.tensor_scalar(out=oh[:], in0=iota_f[:],
                                                   scalar1=qtg[:, rc:rc + 1],
                                                   scalar2=None, op0=ALU.is_equal)
                           ohp = qwk.tile([128, V], f32, tag="ohp")
                           nc.vector.tensor_tensor(out=ohp[:], in0=ql[:],
                                                   in1=oh[:], op=ALU.mult)
                           nc.vector.tensor_reduce(out=tl_all[:, rc:rc + 1],
                                                   in_=ohp[:], op=ALU.add,
                                                   axis=mybir.AxisListType.X)
                           esc = qwk.tile([128, V], f32, tag="esc")
                           nc.scalar.activation(esc[:], ql[:], AF.Exp,
                                                bias=mxn_all[:, rc:rc + 1])
                           nc.vector.tensor_reduce(out=se_all[:, rc:rc + 1],
                                                   in_=esc[:], op=ALU.add,
                                                   axis=mybir.AxisListType.X)

              # ---- final reductions ----
              if DBG_FIN:
               fin = cp.tile([128, RC], f32)
               # lse = ln(se) - mxn ; qterm = (lse - tl)*qmask summed
               nc.scalar.activation(fin[:], se_all[:], AF.Ln)
               nc.any.tensor_sub(fin[:], fin[:], mxn_all[:])
               nc.any.tensor_sub(fin[:], fin[:], tl_all[:])
               scr = cp.tile([128, RC], f32)
               nc.vector.tensor_tensor(out=scr[:], in0=fin[:], in1=qmask[:],
                                       op=ALU.mult)
               nc.vector.tensor_reduce(out=acc[:, 0:1], in_=scr[:], op=ALU.add,
                                       axis=mybir.AxisListType.X)
               # q match: tl + mxn == 0
               nc.any.tensor_add(fin[:], tl_all[:], mxn_all[:])
               nc.vector.tensor_scalar(out=fin[:], in0=fin[:], scalar1=0.0,
                                       scalar2=None, op0=ALU.is_equal)
               nc.vector.tensor_tensor(out=scr[:], in0=fin[:], in1=qmask[:],
                                       op=ALU.mult)
               nc.vector.tensor_reduce(out=acc[:, 2:3], in_=scr[:], op=ALU.add,
                                       axis=mybir.AxisListType.X)

               # p head: reshape pl [1, 2560] -> [128, 20] via DRAM round-trip
               nc.sync.dma_start(out=pl_dram[:], in_=pl_sb[:])
               pl_rm = cp.tile([128, RC], f32)
               nc.sync.dma_start(
                   out=pl_rm[:],
                   in_=pl_dram[0:1, :].rearrange("o (rc p) -> (o p) rc", p=128))
               # softplus(x) = relu(x) + ln(1 + exp(-|x|))
               ab = cp.tile([128, RC], f32)
               nc.scalar.activation(ab[:], pl_rm[:], AF.Abs)
               nc.scalar.activation(ab[:], ab[:], AF.Exp, scale=-1.0)
               nc.scalar.activation(ab[:], ab[:], AF.Ln, bias=1.0)
               rl = cp.tile([128, RC], f32)
               nc.scalar.activation(rl[:], pl_rm[:], AF.Relu)
               nc.any.tensor_add(ab[:], ab[:], rl[:])
               nc.vector.tensor_tensor(out=scr[:], in0=ab[:], in1=pmask[:],
                                       op=ALU.mult)
               nc.vector.tensor_reduce(out=acc[:, 1:2], in_=scr[:], op=ALU.add,
                                       axis=mybir.AxisListType.X)
               nc.vector.tensor_tensor(out=scr[:], in0=pl_rm[:], in1=ptgt[:],
                                       op=ALU.mult)
               nc.vector.tensor_reduce(out=acc[:, 4:5], in_=scr[:], op=ALU.add,
                                       axis=mybir.AxisListType.X)
               # p match: (pl > 0) == ptgt
               gt = cp.tile([128, RC], f32)
               nc.vector.tensor_scalar(out=gt[:], in0=pl_rm[:], scalar1=0.0,
                                       scalar2=None, op0=ALU.is_gt)
               nc.vector.tensor_tensor(out=gt[:], in0=gt[:], in1=ptgt[:],
                                       op=ALU.is_equal)
               nc.vector.tensor_tensor(out=scr[:], in0=gt[:], in1=pmask[:],
                                       op=ALU.mult)
               nc.vector.tensor_reduce(out=acc[:, 3:4], in_=scr[:], op=ALU.add,
                                       axis=mybir.AxisListType.X)

               with tc.tile_pool(name="fps", bufs=1, space="PSUM") as fps:
                   ones32 = cp.tile([128, 1], f32)
                   nc.vector.memset(ones32[:], 1.0)
                   fin_ps = fps.tile([1, 8], f32, space="PSUM")
                   nc.tensor.matmul(fin_ps[:], lhsT=ones32[:], rhs=acc[:],
                                    start=True, stop=True)
                   fin_sb = cp.tile([1, 8], f32)
                   nc.vector.tensor_copy(fin_sb[:], fin_ps[:])
                   nc.sync.dma_start(out=out_d[:], in_=fin_sb[:])

    nc.compile()
    return nc


_NC_CACHE = None
LAST_EXEC_NS = None
LAST_RES = None


def kernel(**inputs):
    global _NC_CACHE
    cores = _host_prep(inputs)
    if _NC_CACHE is None:
        _NC_CACHE = _build_program()
    nc = _NC_CACHE
    in_maps = [{k: np.ascontiguousarray(v) for k, v in cores[c].items()}
               for c in range(NC)]
    trace = os.environ.get("KERNEL_TRACE", "0") == "1"
    res = run_bass_kernel_spmd(nc, in_maps, core_ids=list(range(NC)),
                               trace=trace)
    global LAST_EXEC_NS, LAST_RES
    LAST_EXEC_NS = getattr(res, "exec_time_ns", None)
    LAST_RES = res
    total = np.zeros(8, np.float64)
    for r in res.results:
        total += np.asarray(r["out"], np.float64).reshape(-1)
    q_loss = total[0] / B
    p_loss = (total[1] - total[4]) / B
    q_acc = total[2] / 10240.0
    p_acc = total[3] / (39 * B)
    return np.array([q_loss, p_loss, q_acc, p_acc], np.float32)


if __name__ == "__main__":
    pass

